# revision 4
# baseline (speedup 1.0000x reference)
"""AttentionalGCN forward on 8 Trainium2 NeuronCores.

Math note: the reference's attention block is an exact no-op —
``einsum('ij,ik->ik', softmax(scores), agg) == rowsum(softmax) * agg == agg``
— so the output reduces to

    out = x @ (W_obj + W_skip) + r @ W_rel + A.T @ (x @ W_nobj + b_nobj)
          + (b_obj + b_rel + b_skip)

The A.T @ P term dominates (A is 8192x8192 f32 = 256 MB): this is a
memory-bound streaming matmul. Sharding: core m owns columns
[m*1024, (m+1)*1024) of A (= rows of the output), so no cross-core
reduction is needed; the host concatenates the 8 output shards.

A is 0/1 so it is cast to fp16 on the host (exact, halves DMA bytes).
P is computed on-device in f32 (PSUM) and cast to fp16; the fp16
quantization of P contributes ~7e-5 relative error to the output.
"""

import numpy as np

import concourse.bass as bass
import concourse.bacc as bacc
import concourse.tile as tile
from concourse import mybir
from concourse import bass_utils

N = 8192          # nodes
D = 64            # feature dim
M = 8             # cores
SH = N // M       # 1024 output rows / A columns per core
KT = N // 128     # 64 contraction k-tiles of 128 rows
F16 = mybir.dt.float16
F32 = mybir.dt.float32

_BUILT = {}


def build_bass():
    """One SPMD program, identical on all 8 cores; per-core data differs."""
    nc = bacc.Bacc(
        "TRN2", target_bir_lowering=False, debug=False, num_devices=M
    )
    xT = nc.declare_dram_parameter("xT", [D + 1, N], F16, isOutput=False)
    xmT = nc.declare_dram_parameter("xmT", [D + 1, SH], F16, isOutput=False)
    rT = nc.declare_dram_parameter("rT", [D, SH], F16, isOutput=False)
    a16 = nc.declare_dram_parameter("a16", [N, SH], F16, isOutput=False)
    wnb = nc.declare_dram_parameter("wnb", [D + 1, D], F16, isOutput=False)
    w1 = nc.declare_dram_parameter("w1", [D + 1, D], F16, isOutput=False)
    wrel = nc.declare_dram_parameter("wrel", [D, D], F16, isOutput=False)
    outT = nc.declare_dram_parameter("outT", [D, SH], F32, isOutput=True)

    NC_CHUNKS = 8              # A streamed in 8 chunks of [1024, SH] (2 MB)
    TPC = KT // NC_CHUNKS      # 8 k-tiles per chunk
    NG = KT // 8               # P-phase: 8 k-tiles per PSUM bank group

    with tile.TileContext(nc) as tc:
        with (
            tc.tile_pool(name="const", bufs=1) as const,
            tc.tile_pool(name="abuf", bufs=4) as abuf,
            tc.tile_pool(name="p16pool", bufs=1) as p16pool,
            tc.tile_pool(name="opool", bufs=1) as opool,
            tc.tile_pool(name="psumP", bufs=2, space="PSUM") as psumP,
            tc.tile_pool(name="psumO", bufs=1, space="PSUM") as psumO,
        ):
            xT_sb = const.tile([D + 1, N], F16)
            nc.sync.dma_start(xT_sb[:], xT[:])
            wnb_sb = const.tile([D + 1, D], F16)
            nc.sync.dma_start(wnb_sb[:], wnb[:])
            w1_sb = const.tile([D + 1, D], F16)
            nc.sync.dma_start(w1_sb[:], w1[:])
            wrel_sb = const.tile([D, D], F16)
            nc.sync.dma_start(wrel_sb[:], wrel[:])
            xmT_sb = const.tile([D + 1, SH], F16)
            nc.sync.dma_start(xmT_sb[:], xmT[:])
            rT_sb = const.tile([D, SH], F16)
            nc.sync.dma_start(rT_sb[:], rT[:])

            # P[k] tiles ([128, 64] each) packed as [128, KT*64] fp16
            p16 = p16pool.tile([128, KT * D], F16)

            # A chunks: rows c*1024..(c+1)*1024 -> SBUF [128, TPC, SH]
            a_r = a16.rearrange("(c t p) n -> c p t n", p=128, t=TPC)

            # ---- P phase: P = x_aug @ [W_nobj; b_nobj], f32 in PSUM ----
            for g in range(NG):
                pp = psumP.tile([128, 8 * D], F32)  # one 2KB bank
                for t in range(8):
                    k = g * 8 + t
                    nc.tensor.matmul(
                        pp[:, t * D:(t + 1) * D],
                        xT_sb[:, k * 128:(k + 1) * 128],
                        wnb_sb[:],
                        start=True,
                        stop=True,
                    )
                nc.vector.tensor_copy(p16[:, g * 8 * D:(g + 1) * 8 * D], pp[:])

            # ---- output accumulator [64, 1024] f32 = 2 PSUM banks ----
            po = psumO.tile([D, SH], F32)
            for h in range(2):
                sl = slice(h * 512, (h + 1) * 512)
                # skip+obj projection (bias folded in via ones row of xmT)
                nc.tensor.matmul(
                    po[:, sl], w1_sb[:], xmT_sb[:, sl], start=True, stop=False
                )
                nc.tensor.matmul(
                    po[:, sl], wrel_sb[:], rT_sb[:, sl], start=False, stop=False
                )

            # ---- main phase: po += sum_k P16[k].T-accumulation over A ----
            for c in range(NC_CHUNKS):
                at = abuf.tile([128, TPC, SH], F16)
                nc.sync.dma_start(at[:], a_r[c])
                for t in range(TPC):
                    k = c * TPC + t
                    last = k == KT - 1
                    for h in range(2):
                        sl = slice(h * 512, (h + 1) * 512)
                        nc.tensor.matmul(
                            po[:, sl],
                            p16[:, k * D:(k + 1) * D],
                            at[:, t, sl],
                            start=False,
                            stop=last,
                        )

            out_sb = opool.tile([D, SH], F32)
            nc.vector.tensor_copy(out_sb[:], po[:])
            nc.sync.dma_start(outT[:], out_sb[:])

    nc.compile()
    return nc


def _prep_in_maps(object_features, relationship_features, adjacency_matrix,
                  W_obj, b_obj, W_nobj, b_nobj, W_rel, b_rel,
                  W_skip, b_skip):
    x = np.ascontiguousarray(object_features, dtype=np.float32)
    r = np.ascontiguousarray(relationship_features, dtype=np.float32)
    A = np.asarray(adjacency_matrix, dtype=np.float32)

    ones = np.ones((1, N), np.float32)
    xT16 = np.ascontiguousarray(
        np.concatenate([x.T, ones], axis=0).astype(np.float16))  # [65, N]
    rT16 = np.ascontiguousarray(r.T.astype(np.float16))          # [64, N]

    wnb = np.concatenate([W_nobj, b_nobj[None, :]], axis=0).astype(np.float16)
    w1 = np.concatenate(
        [W_obj + W_skip, (b_obj + b_rel + b_skip)[None, :]], axis=0
    ).astype(np.float16)
    wrel = np.asarray(W_rel, dtype=np.float16)

    in_maps = []
    for m in range(M):
        sl = slice(m * SH, (m + 1) * SH)
        in_maps.append({
            "xT": xT16,
            "xmT": np.ascontiguousarray(xT16[:, sl]),
            "rT": np.ascontiguousarray(rT16[:, sl]),
            "a16": np.ascontiguousarray(A[:, sl].astype(np.float16)),
            "wnb": wnb,
            "w1": w1,
            "wrel": wrel,
        })
    return in_maps


def run(inputs: dict, **run_kwargs):
    """Build (cached), run on cores 0-7, return (output, BassKernelResults)."""
    if "nc" not in _BUILT:
        _BUILT["nc"] = build_bass()
    nc = _BUILT["nc"]
    in_maps = _prep_in_maps(
        inputs["object_features"], inputs["relationship_features"],
        inputs["adjacency_matrix"],
        inputs["W_obj"], inputs["b_obj"], inputs["W_nobj"], inputs["b_nobj"],
        inputs["W_rel"], inputs["b_rel"], inputs["W_skip"], inputs["b_skip"],
    )
    res = bass_utils.run_bass_kernel_spmd(
        nc, in_maps, core_ids=list(range(M)), **run_kwargs
    )
    out = np.concatenate(
        [res.results[m]["outT"].T for m in range(M)], axis=0
    ).astype(np.float32)
    return out, res


def kernel(**inputs) -> np.ndarray:
    out, _ = run(inputs)
    return out


# revision 9
# speedup vs baseline: 1.0382x; 1.0382x over previous
"""AttentionalGCN forward on 8 Trainium2 NeuronCores.

Math note: the reference's attention block is an exact no-op —
``einsum('ij,ik->ik', softmax(scores), agg) == rowsum(softmax) * agg == agg``
— so the output reduces to

    out = x @ (W_obj + W_skip) + r @ W_rel + A.T @ (x @ W_nobj + b_nobj)
          + (b_obj + b_rel + b_skip)

The A.T @ P term dominates (A is 8192x8192 f32 = 256 MB): this is a
memory-bound streaming matmul. Sharding: core m owns columns
[m*1024, (m+1)*1024) of A (= rows of the output), so no cross-core
reduction is needed; the host concatenates the 8 output shards.

A is 0/1 so it is cast to fp16 on the host (exact, halves DMA bytes).
P is computed on-device (fp16 inputs, f32 PSUM accumulate) and cast to
fp16; quantization contributes ~2e-4 relative error to the output.

Raw bacc (no Tile) with hand-placed semaphores: the static dataflow
needs only one wait per instruction, which avoids Tile's preamble and
its ~10 us end-of-kernel drain + all-engine EVSEM barrier.

Per-core plan (core m):
  scalar HWDGE queue: consts (xT [65,8192] f16, weights, xmT, rT)
  sync   HWDGE queue: A column block [8192,1024] f16 in 8 x 2MB chunks
  PE: P-phase   64 matmuls  P[k] = xT_aug[:,kth 128 cols].T @ [Wnb;bnb]
      proj      4  matmuls  po  = [W1;bias].T @ xmT + Wrel.T @ rT
      O-phase  128 matmuls  po += P16[k].T-stationary x A-chunk moving
  DVE: 8 casts PSUM->fp16 (P), final PSUM->SBUF copy
  out: outT [64,1024] f32, host transposes + concatenates
"""

import numpy as np

import concourse.bass as bass
import concourse.bacc as bacc
from concourse import mybir
from concourse import bass_utils

N = 8192          # nodes
D = 64            # feature dim
M = 8             # cores
SH = N // M       # 1024 output rows / A columns per core
KT = N // 128     # 64 contraction k-tiles of 128 rows
F16 = mybir.dt.float16
F32 = mybir.dt.float32

NCH = 8           # A streamed in 8 chunks of [1024, SH] fp16 (2 MB)
TPC = KT // NCH   # 8 k-tiles per chunk
NG = 8            # P-phase groups (8 k-tiles -> one PSUM bank each)
ABUF = 4          # A chunk buffers in SBUF

_BUILT = {}


def build_bass():
    """One SPMD program, identical on all 8 cores; per-core data differs."""
    nc = bacc.Bacc("TRN2", target_bir_lowering=False, debug=False, num_devices=M)

    xT = nc.declare_dram_parameter("xT", [D + 1, N], F16, isOutput=False)
    xmT = nc.declare_dram_parameter("xmT", [D + 1, SH], F16, isOutput=False)
    rT = nc.declare_dram_parameter("rT", [D, SH], F16, isOutput=False)
    a16 = nc.declare_dram_parameter("a16", [N, SH], F16, isOutput=False)
    wnb = nc.declare_dram_parameter("wnb", [D + 1, D], F16, isOutput=False)
    w1 = nc.declare_dram_parameter("w1", [D + 1, D], F16, isOutput=False)
    wrel = nc.declare_dram_parameter("wrel", [D, D], F16, isOutput=False)
    outT = nc.declare_dram_parameter("outT", [D, SH], F32, isOutput=True)

    a_r = a16.rearrange("(c t p) n -> c p t n", p=128, t=TPC)

    from contextlib import ExitStack

    with ExitStack() as ctx:
        xT_sb = ctx.enter_context(nc.sbuf_tensor("xT_sb", [D + 1, N], F16))
        xmT_sb = ctx.enter_context(nc.sbuf_tensor("xmT_sb", [D + 1, SH], F16))
        rT_sb = ctx.enter_context(nc.sbuf_tensor("rT_sb", [D, SH], F16))
        wnb_sb = ctx.enter_context(nc.sbuf_tensor("wnb_sb", [D + 1, D], F16))
        w1_sb = ctx.enter_context(nc.sbuf_tensor("w1_sb", [D + 1, D], F16))
        wrel_sb = ctx.enter_context(nc.sbuf_tensor("wrel_sb", [D, D], F16))
        p16 = ctx.enter_context(nc.sbuf_tensor("p16", [128, KT * D], F16))
        at = ctx.enter_context(nc.sbuf_tensor("at", [128, ABUF, TPC, SH], F16))
        out_sb = ctx.enter_context(nc.sbuf_tensor("out_sb", [D, SH], F32))
        pp = [
            ctx.enter_context(nc.psum_tensor("pp0", [128, 8 * D], F32)),
            ctx.enter_context(nc.psum_tensor("pp1", [128, 8 * D], F32)),
        ]
        po = ctx.enter_context(nc.psum_tensor("po", [D, SH], F32))
        # Per-chunk DMA semaphores: a wait must cover a semaphore's FULL
        # accumulated total (SDMA-engine increments from different DMAs on
        # the same queue interleave, so partial totals are racy).
        dma_cx = ctx.enter_context(nc.semaphore("dma_cx"))  # xT+wnb
        dma_cw = ctx.enter_context(nc.semaphore("dma_cw"))  # w1/wrel/xmT/rT
        dma_a = [
            ctx.enter_context(nc.semaphore(f"dma_a{c}")) for c in range(NCH)
        ]
        pe_p = ctx.enter_context(nc.semaphore("pe_p"))    # P group done
        dve_p = ctx.enter_context(nc.semaphore("dve_p"))  # P cast done
        pe_c = ctx.enter_context(nc.semaphore("pe_c"))    # O chunk done
        dve_o = ctx.enter_context(nc.semaphore("dve_o"))  # final copy done
        dma_o = ctx.enter_context(nc.semaphore("dma_o"))  # output DMA done
        block = ctx.enter_context(nc.Block(no_gpsimd_drain=True))

        @block.scalar
        def _(scalar):
            scalar.dma_start(xT_sb[:], xT[:]).then_inc(dma_cx, 16)
            scalar.dma_start(wnb_sb[:], wnb[:]).then_inc(dma_cx, 16)
            scalar.dma_start(w1_sb[:], w1[:]).then_inc(dma_cw, 16)
            scalar.dma_start(wrel_sb[:], wrel[:]).then_inc(dma_cw, 16)
            scalar.dma_start(xmT_sb[:], xmT[:]).then_inc(dma_cw, 16)
            scalar.dma_start(rT_sb[:], rT[:]).then_inc(dma_cw, 16)

        @block.sync
        def _(sync):
            for c in range(NCH):
                if c >= ABUF:
                    sync.wait_ge(pe_c, c - ABUF + 1)
                sync.dma_start(at[:, c % ABUF], a_r[c]).then_inc(dma_a[c], 16)
            sync.wait_ge(dve_o, 1)
            sync.dma_start(outT[:], out_sb[:]).then_inc(dma_o, 16)
            sync.wait_ge(dma_o, 16)

        @block.tensor
        def _(tensor):
            # ---- P phase: P = x_aug @ [W_nobj; b_nobj] (f32 in PSUM) ----
            tensor.wait_ge(dma_cx, 32)          # xT + wnb landed
            for g in range(NG):
                if g >= 2:
                    tensor.wait_ge(dve_p, g - 1)  # bank g%2 cast done
                for t in range(8):
                    k = g * 8 + t
                    mm = tensor.matmul(
                        pp[g % 2][:, t * D:(t + 1) * D],
                        xT_sb[:, k * 128:(k + 1) * 128],
                        wnb_sb[:],
                        start=True,
                        stop=True,
                    )
                mm.then_inc(pe_p, 1)

            # ---- projections (biases folded via ones rows) ----
            tensor.wait_ge(dma_cw, 64)          # w1/wrel/xmT/rT landed
            for h in range(2):
                sl = slice(h * 512, (h + 1) * 512)
                tensor.matmul(po[:, sl], w1_sb[:], xmT_sb[:, sl],
                              start=True, stop=False)
                tensor.matmul(po[:, sl], wrel_sb[:], rT_sb[:, sl],
                              start=False, stop=False)

            # ---- O phase: po += sum_k P16[k] x A ----
            tensor.wait_ge(dve_p, NG)           # all of P16 ready
            for c in range(NCH):
                tensor.wait_ge(dma_a[c], 16)
                for t in range(TPC):
                    k = c * TPC + t
                    last = k == KT - 1
                    for h in range(2):
                        sl = slice(h * 512, (h + 1) * 512)
                        mm = tensor.matmul(
                            po[:, sl],
                            p16[:, k * D:(k + 1) * D],
                            at[:, c % ABUF, t, sl],
                            start=False,
                            stop=last,
                        )
                mm.then_inc(pe_c, 1)

        @block.vector
        def _(vector):
            for g in range(NG):
                vector.wait_ge(pe_p, g + 1)
                vector.tensor_copy(
                    p16[:, g * 8 * D:(g + 1) * 8 * D], pp[g % 2][:]
                ).then_inc(dve_p, 1)
            vector.wait_ge(pe_c, NCH)
            vector.tensor_copy(out_sb[:], po[:]).then_inc(dve_o, 1)

    nc.compile()
    return nc


def _prep_in_maps(object_features, relationship_features, adjacency_matrix,
                  W_obj, b_obj, W_nobj, b_nobj, W_rel, b_rel,
                  W_skip, b_skip):
    x = np.ascontiguousarray(object_features, dtype=np.float32)
    r = np.ascontiguousarray(relationship_features, dtype=np.float32)
    A = np.asarray(adjacency_matrix, dtype=np.float32)

    ones = np.ones((1, N), np.float32)
    xT16 = np.ascontiguousarray(
        np.concatenate([x.T, ones], axis=0).astype(np.float16))  # [65, N]
    rT16 = np.ascontiguousarray(r.T.astype(np.float16))          # [64, N]

    wnb = np.concatenate([W_nobj, b_nobj[None, :]], axis=0).astype(np.float16)
    w1 = np.concatenate(
        [W_obj + W_skip, (b_obj + b_rel + b_skip)[None, :]], axis=0
    ).astype(np.float16)
    wrel = np.asarray(W_rel, dtype=np.float16)

    in_maps = []
    for m in range(M):
        sl = slice(m * SH, (m + 1) * SH)
        in_maps.append({
            "xT": xT16,
            "xmT": np.ascontiguousarray(xT16[:, sl]),
            "rT": np.ascontiguousarray(rT16[:, sl]),
            "a16": np.ascontiguousarray(A[:, sl].astype(np.float16)),
            "wnb": wnb,
            "w1": w1,
            "wrel": wrel,
        })
    return in_maps


def run(inputs: dict, **run_kwargs):
    """Build (cached), run on cores 0-7, return (output, BassKernelResults)."""
    if "nc" not in _BUILT:
        _BUILT["nc"] = build_bass()
    nc = _BUILT["nc"]
    in_maps = _prep_in_maps(
        inputs["object_features"], inputs["relationship_features"],
        inputs["adjacency_matrix"],
        inputs["W_obj"], inputs["b_obj"], inputs["W_nobj"], inputs["b_nobj"],
        inputs["W_rel"], inputs["b_rel"], inputs["W_skip"], inputs["b_skip"],
    )
    res = bass_utils.run_bass_kernel_spmd(
        nc, in_maps, core_ids=list(range(M)), **run_kwargs
    )
    out = np.concatenate(
        [res.results[m]["outT"].T for m in range(M)], axis=0
    ).astype(np.float32)
    return out, res


def kernel(**inputs) -> np.ndarray:
    out, _ = run(inputs)
    return out


# revision 10
# speedup vs baseline: 1.0541x; 1.0153x over previous
"""AttentionalGCN forward on 8 Trainium2 NeuronCores.

Math note: the reference's attention block is an exact no-op —
``einsum('ij,ik->ik', softmax(scores), agg) == rowsum(softmax) * agg == agg``
— so the output reduces to

    out = x @ (W_obj + W_skip) + r @ W_rel + A.T @ (x @ W_nobj + b_nobj)
          + (b_obj + b_rel + b_skip)

The A.T @ P term dominates (A is 8192x8192 f32 = 256 MB): this is a
memory-bound streaming matmul. Sharding: core m owns columns
[m*1024, (m+1)*1024) of A (= rows of the output), so no cross-core
reduction is needed; the host concatenates the 8 output shards.

A is 0/1 so it is cast to fp16 on the host (exact, halves DMA bytes)
and pre-tiled so each (partition, chunk) run is one contiguous 8 KB
DMA descriptor. P is computed on-device (fp16 inputs, f32 PSUM
accumulate) and cast to fp16; quantization adds ~2e-4 relative error.

Raw bacc (no Tile) with hand-placed semaphores: the static dataflow
needs only one wait per instruction, which avoids Tile's preamble and
its ~10 us end-of-kernel drain + all-engine EVSEM barrier. All input
DMAs ride one HWDGE ring in explicit order (xT first — both "sync"
and "scalar" HWDGE paths share the physical ring, so a second queue
does not parallelize). Every DMA gets its own semaphore; a wait must
cover a semaphore's FULL accumulated total (per-SDMA-engine
increments from different DMAs interleave, partial totals are racy).

Per-core plan (core m):
  PE: P-phase   64 matmuls  P[k] = xT_aug[:,kth 128 cols].T @ [Wnb;bnb]
      proj      4  matmuls  po  = [W1;bias].T @ xmT + Wrel.T @ rT
      O-phase  128 matmuls  po += P16[k] stationary x A-chunk moving
  DVE: 8 casts PSUM->fp16 (P), final PSUM->SBUF copies (2 halves)
  out: outT [64,1024] f32, host transposes + concatenates
"""

from contextlib import ExitStack

import numpy as np

import concourse.bass as bass
import concourse.bacc as bacc
from concourse import mybir
from concourse import bass_utils

N = 8192          # nodes
D = 64            # feature dim
M = 8             # cores
SH = N // M       # 1024 output rows / A columns per core
KT = N // 128     # 64 contraction k-tiles of 128 rows
F16 = mybir.dt.float16
F32 = mybir.dt.float32

NCH = 16          # A streamed in 16 chunks of 4 k-tiles (1 MB fp16)
TPC = KT // NCH   # 4 k-tiles per chunk
NG = 8            # P-phase groups (8 k-tiles -> one PSUM bank each)
ABUF = 8          # A chunk buffers in SBUF

_BUILT = {}


def build_bass():
    """One SPMD program, identical on all 8 cores; per-core data differs."""
    nc = bacc.Bacc("TRN2", target_bir_lowering=False, debug=False, num_devices=M)

    xT = nc.declare_dram_parameter("xT", [D + 1, N], F16, isOutput=False)
    xmT = nc.declare_dram_parameter("xmT", [D + 1, SH], F16, isOutput=False)
    rT = nc.declare_dram_parameter("rT", [D, SH], F16, isOutput=False)
    # host pre-tiled: row p*KT + k holds A[k*128 + p, :] of this core's block
    a16 = nc.declare_dram_parameter("a16", [N, SH], F16, isOutput=False)
    wnb = nc.declare_dram_parameter("wnb", [D + 1, D], F16, isOutput=False)
    w1 = nc.declare_dram_parameter("w1", [D + 1, D], F16, isOutput=False)
    wrel = nc.declare_dram_parameter("wrel", [D, D], F16, isOutput=False)
    outT = nc.declare_dram_parameter("outT", [D, SH], F32, isOutput=True)

    # [p, c, t, n]: chunk c for partition p is one contiguous TPC*SH run
    a_r = a16.rearrange("(p c t) n -> c p (t n)", p=128, c=NCH, t=TPC)

    with ExitStack() as ctx:
        xT_sb = ctx.enter_context(nc.sbuf_tensor("xT_sb", [D + 1, N], F16))
        xmT_sb = ctx.enter_context(nc.sbuf_tensor("xmT_sb", [D + 1, SH], F16))
        rT_sb = ctx.enter_context(nc.sbuf_tensor("rT_sb", [D, SH], F16))
        wnb_sb = ctx.enter_context(nc.sbuf_tensor("wnb_sb", [D + 1, D], F16))
        w1_sb = ctx.enter_context(nc.sbuf_tensor("w1_sb", [D + 1, D], F16))
        wrel_sb = ctx.enter_context(nc.sbuf_tensor("wrel_sb", [D, D], F16))
        p16 = ctx.enter_context(nc.sbuf_tensor("p16", [128, KT * D], F16))
        at = ctx.enter_context(
            nc.sbuf_tensor("at", [128, ABUF, TPC * SH], F16))
        out_sb = ctx.enter_context(nc.sbuf_tensor("out_sb", [D, SH], F32))
        pp = [
            ctx.enter_context(nc.psum_tensor("pp0", [128, 8 * D], F32)),
            ctx.enter_context(nc.psum_tensor("pp1", [128, 8 * D], F32)),
        ]
        po = ctx.enter_context(nc.psum_tensor("po", [D, SH], F32))

        dma_xt = ctx.enter_context(nc.semaphore("dma_xt"))  # xT + wnb
        dma_cw = ctx.enter_context(nc.semaphore("dma_cw"))  # w1/wrel/xmT/rT
        dma_a = [
            ctx.enter_context(nc.semaphore(f"dma_a{c}")) for c in range(NCH)
        ]
        pe_p = ctx.enter_context(nc.semaphore("pe_p"))    # P group done
        dve_p = ctx.enter_context(nc.semaphore("dve_p"))  # P cast done
        pe_c = ctx.enter_context(nc.semaphore("pe_c"))    # O chunk done
        pe_h0 = ctx.enter_context(nc.semaphore("pe_h0"))  # last chunk h=0
        dve_o = ctx.enter_context(nc.semaphore("dve_o"))  # out copy halves
        dma_o = ctx.enter_context(nc.semaphore("dma_o"))  # output DMA done
        block = ctx.enter_context(nc.Block(no_gpsimd_drain=True))

        @block.sync
        def _(sync):
            # one HWDGE ring, explicit order: xT first (P-phase dep), then
            # the tiny consts, then the A stream.
            sync.dma_start(xT_sb[:], xT[:]).then_inc(dma_xt, 16)
            sync.dma_start(wnb_sb[:], wnb[:]).then_inc(dma_xt, 16)
            sync.dma_start(w1_sb[:], w1[:]).then_inc(dma_cw, 16)
            sync.dma_start(wrel_sb[:], wrel[:]).then_inc(dma_cw, 16)
            sync.dma_start(xmT_sb[:], xmT[:]).then_inc(dma_cw, 16)
            sync.dma_start(rT_sb[:], rT[:]).then_inc(dma_cw, 16)
            for c in range(NCH):
                if c >= ABUF:
                    sync.wait_ge(pe_c, c - ABUF + 1)
                sync.dma_start(at[:, c % ABUF], a_r[c]).then_inc(dma_a[c], 16)
            # output, split in halves so h=0 streams while h=1 finishes
            sync.wait_ge(dve_o, 1)
            sync.dma_start(outT[:, 0:512], out_sb[:, 0:512]).then_inc(dma_o, 16)
            sync.wait_ge(dve_o, 2)
            sync.dma_start(outT[:, 512:1024], out_sb[:, 512:1024]).then_inc(
                dma_o, 16)
            sync.wait_ge(dma_o, 32)

        @block.tensor
        def _(tensor):
            # ---- P phase: P = x_aug @ [W_nobj; b_nobj] (f32 in PSUM) ----
            tensor.wait_ge(dma_xt, 32)          # xT + wnb landed
            for g in range(NG):
                if g >= 2:
                    tensor.wait_ge(dve_p, g - 1)  # bank g%2 cast done
                for t in range(8):
                    k = g * 8 + t
                    mm = tensor.matmul(
                        pp[g % 2][:, t * D:(t + 1) * D],
                        xT_sb[:, k * 128:(k + 1) * 128],
                        wnb_sb[:],
                        start=True,
                        stop=True,
                    )
                mm.then_inc(pe_p, 1)

            # ---- projections (biases folded via ones rows) ----
            tensor.wait_ge(dma_cw, 64)          # w1/wrel/xmT/rT landed
            for h in range(2):
                sl = slice(h * 512, (h + 1) * 512)
                tensor.matmul(po[:, sl], w1_sb[:], xmT_sb[:, sl],
                              start=True, stop=False)
                tensor.matmul(po[:, sl], wrel_sb[:], rT_sb[:, sl],
                              start=False, stop=False)

            # ---- O phase: po += sum_k P16[k] x A ----
            tensor.wait_ge(dve_p, NG)           # all of P16 ready
            for c in range(NCH):
                tensor.wait_ge(dma_a[c], 16)
                last_c = c == NCH - 1
                # last chunk h-major so half 0 finishes first
                loops = ([(h, t) for h in range(2) for t in range(TPC)]
                         if last_c else
                         [(h, t) for t in range(TPC) for h in range(2)])
                for i, (h, t) in enumerate(loops):
                    k = c * TPC + t
                    sl = slice(h * 512, (h + 1) * 512)
                    mm = tensor.matmul(
                        po[:, sl],
                        p16[:, k * D:(k + 1) * D],
                        at[:, c % ABUF, t * SH + h * 512:t * SH + h * 512 + 512],
                        start=False,
                        stop=last_c and t == TPC - 1,
                    )
                    if last_c and h == 0 and t == TPC - 1:
                        mm.then_inc(pe_h0, 1)
                mm.then_inc(pe_c, 1)

        @block.vector
        def _(vector):
            for g in range(NG):
                vector.wait_ge(pe_p, g + 1)
                vector.tensor_copy(
                    p16[:, g * 8 * D:(g + 1) * 8 * D], pp[g % 2][:]
                ).then_inc(dve_p, 1)
            vector.wait_ge(pe_h0, 1)
            vector.tensor_copy(out_sb[:, 0:512], po[:, 0:512]).then_inc(
                dve_o, 1)
            vector.wait_ge(pe_c, NCH)
            vector.tensor_copy(out_sb[:, 512:1024], po[:, 512:1024]).then_inc(
                dve_o, 1)

    nc.compile()
    return nc


def _prep_in_maps(object_features, relationship_features, adjacency_matrix,
                  W_obj, b_obj, W_nobj, b_nobj, W_rel, b_rel,
                  W_skip, b_skip):
    x = np.ascontiguousarray(object_features, dtype=np.float32)
    r = np.ascontiguousarray(relationship_features, dtype=np.float32)
    A = np.asarray(adjacency_matrix, dtype=np.float32)

    ones = np.ones((1, N), np.float32)
    xT16 = np.ascontiguousarray(
        np.concatenate([x.T, ones], axis=0).astype(np.float16))  # [65, N]
    rT16 = np.ascontiguousarray(r.T.astype(np.float16))          # [64, N]

    wnb = np.concatenate([W_nobj, b_nobj[None, :]], axis=0).astype(np.float16)
    w1 = np.concatenate(
        [W_obj + W_skip, (b_obj + b_rel + b_skip)[None, :]], axis=0
    ).astype(np.float16)
    wrel = np.asarray(W_rel, dtype=np.float16)

    in_maps = []
    for m in range(M):
        sl = slice(m * SH, (m + 1) * SH)
        # pre-tile the A block: row p*KT + k  <-  A[k*128 + p, sl]
        blk = A[:, sl].astype(np.float16)            # [8192, 1024]
        blk = np.ascontiguousarray(
            blk.reshape(KT, 128, SH).transpose(1, 0, 2).reshape(N, SH))
        in_maps.append({
            "xT": xT16,
            "xmT": np.ascontiguousarray(xT16[:, sl]),
            "rT": np.ascontiguousarray(rT16[:, sl]),
            "a16": blk,
            "wnb": wnb,
            "w1": w1,
            "wrel": wrel,
        })
    return in_maps


def run(inputs: dict, **run_kwargs):
    """Build (cached), run on cores 0-7, return (output, BassKernelResults)."""
    if "nc" not in _BUILT:
        _BUILT["nc"] = build_bass()
    nc = _BUILT["nc"]
    in_maps = _prep_in_maps(
        inputs["object_features"], inputs["relationship_features"],
        inputs["adjacency_matrix"],
        inputs["W_obj"], inputs["b_obj"], inputs["W_nobj"], inputs["b_nobj"],
        inputs["W_rel"], inputs["b_rel"], inputs["W_skip"], inputs["b_skip"],
    )
    res = bass_utils.run_bass_kernel_spmd(
        nc, in_maps, core_ids=list(range(M)), **run_kwargs
    )
    out = np.concatenate(
        [res.results[m]["outT"].T for m in range(M)], axis=0
    ).astype(np.float32)
    return out, res


def kernel(**inputs) -> np.ndarray:
    out, _ = run(inputs)
    return out


# revision 11
# speedup vs baseline: 1.1303x; 1.0723x over previous
"""AttentionalGCN forward on 8 Trainium2 NeuronCores.

Math note: the reference's attention block is an exact no-op —
``einsum('ij,ik->ik', softmax(scores), agg) == rowsum(softmax) * agg == agg``
— so the output reduces to

    out = x @ (W_obj + W_skip) + r @ W_rel + A.T @ (x @ W_nobj + b_nobj)
          + (b_obj + b_rel + b_skip)

The A.T @ P term dominates (A is 8192x8192 f32 = 256 MB): this is a
memory-bound streaming matmul. Sharding: core m owns columns
[m*1024, (m+1)*1024) of A (= rows of the output), so no cross-core
reduction is needed; the host concatenates the 8 output shards.

A is 0/1 so it is cast to fp16 on the host (exact, halves DMA bytes)
and pre-tiled so each (partition, chunk) run is one contiguous 8 KB
DMA descriptor. P is computed on-device (fp16 inputs, f32 PSUM
accumulate) and cast to fp16; quantization adds ~2e-4 relative error.

Raw bacc (no Tile) with hand-placed semaphores: the static dataflow
needs only one wait per instruction, which avoids Tile's preamble and
its ~10 us end-of-kernel drain + all-engine EVSEM barrier. All input
DMAs ride one HWDGE ring in explicit order (xT first — both "sync"
and "scalar" HWDGE paths share the physical ring, so a second queue
does not parallelize). Every DMA gets its own semaphore; a wait must
cover a semaphore's FULL accumulated total (per-SDMA-engine
increments from different DMAs interleave, partial totals are racy).

Per-core plan (core m):
  PE: P-phase   64 matmuls  P[k] = xT_aug[:,kth 128 cols].T @ [Wnb;bnb]
      proj      4  matmuls  po  = [W1;bias].T @ xmT + Wrel.T @ rT
      O-phase  128 matmuls  po += P16[k] stationary x A-chunk moving
  DVE: 8 casts PSUM->fp16 (P), final PSUM->SBUF copies (2 halves)
  out: outT [64,1024] f32, host transposes + concatenates
"""

from contextlib import ExitStack

import numpy as np

import concourse.bass as bass
import concourse.bacc as bacc
from concourse import mybir
from concourse import bass_utils

N = 8192          # nodes
D = 64            # feature dim
M = 8             # cores
SH = N // M       # 1024 output rows / A columns per core
KT = N // 128     # 64 contraction k-tiles of 128 rows
F16 = mybir.dt.float16
F32 = mybir.dt.float32

NCH = 16          # A streamed in 16 chunks of 4 k-tiles (1 MB fp16)
TPC = KT // NCH   # 4 k-tiles per chunk
NG = 8            # P-phase groups (8 k-tiles -> one PSUM bank each)
ABUF = 8          # A chunk buffers in SBUF

_BUILT = {}


def build_bass():
    """One SPMD program, identical on all 8 cores; per-core data differs."""
    nc = bacc.Bacc("TRN2", target_bir_lowering=False, debug=False, num_devices=M)

    xT = nc.declare_dram_parameter("xT", [D + 1, N], F16, isOutput=False)
    xmT = nc.declare_dram_parameter("xmT", [D + 1, SH], F16, isOutput=False)
    rT = nc.declare_dram_parameter("rT", [D, SH], F16, isOutput=False)
    # host pre-tiled: row p*KT + k holds A[k*128 + p, :] of this core's block
    a16 = nc.declare_dram_parameter("a16", [N, SH], F16, isOutput=False)
    wnb = nc.declare_dram_parameter("wnb", [D + 1, D], F16, isOutput=False)
    w1 = nc.declare_dram_parameter("w1", [D + 1, D], F16, isOutput=False)
    wrel = nc.declare_dram_parameter("wrel", [D, D], F16, isOutput=False)
    outT = nc.declare_dram_parameter("outT", [D, SH], F32, isOutput=True)

    # [p, c, t, n]: chunk c for partition p is one contiguous TPC*SH run
    a_r = a16.rearrange("(p c t) n -> c p (t n)", p=128, c=NCH, t=TPC)

    with ExitStack() as ctx:
        xT_sb = ctx.enter_context(nc.sbuf_tensor("xT_sb", [D + 1, N], F16))
        xmT_sb = ctx.enter_context(nc.sbuf_tensor("xmT_sb", [D + 1, SH], F16))
        rT_sb = ctx.enter_context(nc.sbuf_tensor("rT_sb", [D, SH], F16))
        wnb_sb = ctx.enter_context(nc.sbuf_tensor("wnb_sb", [D + 1, D], F16))
        w1_sb = ctx.enter_context(nc.sbuf_tensor("w1_sb", [D + 1, D], F16))
        wrel_sb = ctx.enter_context(nc.sbuf_tensor("wrel_sb", [D, D], F16))
        p16 = ctx.enter_context(nc.sbuf_tensor("p16", [128, KT * D], F16))
        at = ctx.enter_context(
            nc.sbuf_tensor("at", [128, ABUF, TPC * SH], F16))
        out_sb = ctx.enter_context(nc.sbuf_tensor("out_sb", [D, SH], F32))
        pp = [
            ctx.enter_context(nc.psum_tensor("pp0", [128, 8 * D], F32)),
            ctx.enter_context(nc.psum_tensor("pp1", [128, 8 * D], F32)),
        ]
        po = ctx.enter_context(nc.psum_tensor("po", [D, SH], F32))

        dma_xt = ctx.enter_context(nc.semaphore("dma_xt"))  # xT + wnb
        dma_cw = ctx.enter_context(nc.semaphore("dma_cw"))  # w1/wrel/xmT/rT
        dma_a = [
            ctx.enter_context(nc.semaphore(f"dma_a{c}")) for c in range(NCH)
        ]
        pe_p = ctx.enter_context(nc.semaphore("pe_p"))    # P group done
        dve_p = ctx.enter_context(nc.semaphore("dve_p"))  # P cast done
        pe_c = ctx.enter_context(nc.semaphore("pe_c"))    # O chunk done
        pe_h0 = ctx.enter_context(nc.semaphore("pe_h0"))  # last chunk h=0
        dve_o = ctx.enter_context(nc.semaphore("dve_o"))  # out copy halves
        dma_o = ctx.enter_context(nc.semaphore("dma_o"))  # output DMA done
        block = ctx.enter_context(nc.Block(no_gpsimd_drain=True))

        @block.gpsimd
        def _(gpsimd):
            # consts ride the SWDGE ring so the HWDGE ring is pure A stream
            # (xT has 65 partitions -> only ~half the SDMA engines serve it;
            # on the A ring it would add ~5 us of serial time).
            gpsimd.dma_start(xT_sb[:], xT[:]).then_inc(dma_xt, 16)
            gpsimd.dma_start(wnb_sb[:], wnb[:]).then_inc(dma_xt, 16)
            gpsimd.dma_start(w1_sb[:], w1[:]).then_inc(dma_cw, 16)
            gpsimd.dma_start(wrel_sb[:], wrel[:]).then_inc(dma_cw, 16)
            gpsimd.dma_start(xmT_sb[:], xmT[:]).then_inc(dma_cw, 16)
            gpsimd.dma_start(rT_sb[:], rT[:]).then_inc(dma_cw, 16)

        @block.sync
        def _(sync):
            for c in range(NCH):
                if c >= ABUF:
                    sync.wait_ge(pe_c, c - ABUF + 1)
                sync.dma_start(at[:, c % ABUF], a_r[c]).then_inc(dma_a[c], 16)
            # output, split in halves so h=0 streams while h=1 finishes
            sync.wait_ge(dve_o, 1)
            sync.dma_start(outT[:, 0:512], out_sb[:, 0:512]).then_inc(dma_o, 16)
            sync.wait_ge(dve_o, 2)
            sync.dma_start(outT[:, 512:1024], out_sb[:, 512:1024]).then_inc(
                dma_o, 16)
            sync.wait_ge(dma_o, 32)

        @block.tensor
        def _(tensor):
            # ---- P phase: P = x_aug @ [W_nobj; b_nobj] (f32 in PSUM) ----
            tensor.wait_ge(dma_xt, 32)          # xT + wnb landed
            for g in range(NG):
                if g >= 2:
                    tensor.wait_ge(dve_p, g - 1)  # bank g%2 cast done
                for t in range(8):
                    k = g * 8 + t
                    mm = tensor.matmul(
                        pp[g % 2][:, t * D:(t + 1) * D],
                        xT_sb[:, k * 128:(k + 1) * 128],
                        wnb_sb[:],
                        start=True,
                        stop=True,
                    )
                mm.then_inc(pe_p, 1)

            # ---- projections (biases folded via ones rows) ----
            tensor.wait_ge(dma_cw, 64)          # w1/wrel/xmT/rT landed
            for h in range(2):
                sl = slice(h * 512, (h + 1) * 512)
                tensor.matmul(po[:, sl], w1_sb[:], xmT_sb[:, sl],
                              start=True, stop=False)
                tensor.matmul(po[:, sl], wrel_sb[:], rT_sb[:, sl],
                              start=False, stop=False)

            # ---- O phase: po += sum_k P16[k] x A ----
            tensor.wait_ge(dve_p, NG)           # all of P16 ready
            for c in range(NCH):
                tensor.wait_ge(dma_a[c], 16)
                last_c = c == NCH - 1
                # last chunk h-major so half 0 finishes first
                loops = ([(h, t) for h in range(2) for t in range(TPC)]
                         if last_c else
                         [(h, t) for t in range(TPC) for h in range(2)])
                for i, (h, t) in enumerate(loops):
                    k = c * TPC + t
                    sl = slice(h * 512, (h + 1) * 512)
                    mm = tensor.matmul(
                        po[:, sl],
                        p16[:, k * D:(k + 1) * D],
                        at[:, c % ABUF, t * SH + h * 512:t * SH + h * 512 + 512],
                        start=False,
                        stop=last_c and t == TPC - 1,
                    )
                    if last_c and h == 0 and t == TPC - 1:
                        mm.then_inc(pe_h0, 1)
                mm.then_inc(pe_c, 1)

        @block.vector
        def _(vector):
            for g in range(NG):
                vector.wait_ge(pe_p, g + 1)
                vector.tensor_copy(
                    p16[:, g * 8 * D:(g + 1) * 8 * D], pp[g % 2][:]
                ).then_inc(dve_p, 1)
            vector.wait_ge(pe_h0, 1)
            vector.tensor_copy(out_sb[:, 0:512], po[:, 0:512]).then_inc(
                dve_o, 1)
            vector.wait_ge(pe_c, NCH)
            vector.tensor_copy(out_sb[:, 512:1024], po[:, 512:1024]).then_inc(
                dve_o, 1)

    nc.compile()
    return nc


def _prep_in_maps(object_features, relationship_features, adjacency_matrix,
                  W_obj, b_obj, W_nobj, b_nobj, W_rel, b_rel,
                  W_skip, b_skip):
    x = np.ascontiguousarray(object_features, dtype=np.float32)
    r = np.ascontiguousarray(relationship_features, dtype=np.float32)
    A = np.asarray(adjacency_matrix, dtype=np.float32)

    ones = np.ones((1, N), np.float32)
    xT16 = np.ascontiguousarray(
        np.concatenate([x.T, ones], axis=0).astype(np.float16))  # [65, N]
    rT16 = np.ascontiguousarray(r.T.astype(np.float16))          # [64, N]

    wnb = np.concatenate([W_nobj, b_nobj[None, :]], axis=0).astype(np.float16)
    w1 = np.concatenate(
        [W_obj + W_skip, (b_obj + b_rel + b_skip)[None, :]], axis=0
    ).astype(np.float16)
    wrel = np.asarray(W_rel, dtype=np.float16)

    in_maps = []
    for m in range(M):
        sl = slice(m * SH, (m + 1) * SH)
        # pre-tile the A block: row p*KT + k  <-  A[k*128 + p, sl]
        blk = A[:, sl].astype(np.float16)            # [8192, 1024]
        blk = np.ascontiguousarray(
            blk.reshape(KT, 128, SH).transpose(1, 0, 2).reshape(N, SH))
        in_maps.append({
            "xT": xT16,
            "xmT": np.ascontiguousarray(xT16[:, sl]),
            "rT": np.ascontiguousarray(rT16[:, sl]),
            "a16": blk,
            "wnb": wnb,
            "w1": w1,
            "wrel": wrel,
        })
    return in_maps


def run(inputs: dict, **run_kwargs):
    """Build (cached), run on cores 0-7, return (output, BassKernelResults)."""
    if "nc" not in _BUILT:
        _BUILT["nc"] = build_bass()
    nc = _BUILT["nc"]
    in_maps = _prep_in_maps(
        inputs["object_features"], inputs["relationship_features"],
        inputs["adjacency_matrix"],
        inputs["W_obj"], inputs["b_obj"], inputs["W_nobj"], inputs["b_nobj"],
        inputs["W_rel"], inputs["b_rel"], inputs["W_skip"], inputs["b_skip"],
    )
    res = bass_utils.run_bass_kernel_spmd(
        nc, in_maps, core_ids=list(range(M)), **run_kwargs
    )
    out = np.concatenate(
        [res.results[m]["outT"].T for m in range(M)], axis=0
    ).astype(np.float32)
    return out, res


def kernel(**inputs) -> np.ndarray:
    out, _ = run(inputs)
    return out
